# revision 21
# baseline (speedup 1.0000x reference)
"""Multi-head self-attention with RoPE on 8 Trainium2 NeuronCores.

Sharding: tensor-parallel over the 16 heads (2 heads per core) for the
QKV projections + attention, then an AllToAll that re-shards by token so
each core runs the output projection for its 512-token block.

QKV/Wo projections run as float32r (full-rate fp32 on the PE array,
~1e-4 rel); the attention matmuls (QK^T and PV) run in bf16 so the PE
array stays dense (fast FWL weight loads) and hot. Softmax skips the
max-subtraction (scores/8 stay in [-8, 8] for inputs with unit-variance
activations and 1/sqrt(E)-scaled weights) and gets its denominators for
free from an appended ones-row in the PV matmul. RoPE cos/sin come from
the integer positions with a Cody-Waite range reduction + the ACT
engine's Sin spline.
"""

import sys

for _p in ("/opt/trn_rl_repo", "/opt/pypackages"):
    if _p not in sys.path:
        sys.path.append(_p)

import numpy as np

import concourse.bass as bass
import concourse.mybir as mybir
import concourse.tile as tile
from concourse.bass_utils import run_bass_kernel_spmd
import bass_rust

A = mybir.AluOpType
F32 = mybir.dt.float32
F32R = mybir.dt.float32r
BF16 = mybir.dt.bfloat16
AF = mybir.ActivationFunctionType

B, S, E, H, D = 2, 2048, 1024, 16, 64
NT = B * S            # 4096 tokens, batch-major
NCORES = 8

TWO_PI = 2 * np.pi
INV2PI = float(np.float32(1.0 / TWO_PI))
MAGIC = 12582912.0    # 1.5 * 2^23: add+sub rounds fp32 to nearest int
C1 = 6.28125          # 2*pi split: C1 exact in fp32 with short mantissa
C2 = float(np.float32(TWO_PI - C1))
PI = float(np.pi)
HALF_PI = float(np.pi / 2)

# pair order: head-major so each head's two batches finish together
PAIR_BH = [(0, 0), (1, 0), (0, 1), (1, 1)]


def _split_multisync(nc, max_waits=1, max_updates=1):
    """This container's walrus accepts at most one sync-wait and one
    sync-update per instruction; split extras onto adjacent NoOps."""
    ctr = 0
    for f in nc.m.functions:
        for bb in f.blocks:
            new_list = []
            changed = False
            for ins in bb.instructions:
                si = ins.sync_info
                pre, post = [], []
                if si is not None:
                    waits = list(si.on_wait) if si.on_wait else []
                    if len(waits) > max_waits:
                        for w in waits[:-max_waits]:
                            ctr += 1
                            nop = bass_rust.InstNoOp(
                                name=f"I-mws-{ctr}", ins=[], outs=[])
                            nop.engine = ins.engine
                            nop.sync_info = bass_rust.SyncInfo(
                                on_wait=[w], on_update=[])
                            pre.append(nop)
                        si.on_wait = waits[-max_waits:]
                    upds = list(si.on_update) if si.on_update else []
                    if len(upds) > max_updates:
                        si.on_update = upds[:max_updates]
                        for u in upds[max_updates:]:
                            ctr += 1
                            nop = bass_rust.InstNoOp(
                                name=f"I-mus-{ctr}", ins=[], outs=[])
                            nop.engine = ins.engine
                            nop.sync_info = bass_rust.SyncInfo(
                                on_wait=[], on_update=[u])
                            post.append(nop)
                if pre or post:
                    changed = True
                new_list.extend(pre)
                new_list.append(ins)
                new_list.extend(post)
            if changed:
                bb.instructions = new_list


def _build_nc():
    nc = bass.Bass()

    xT = nc.declare_dram_parameter("xT", [E, NT], F32R, isOutput=False)
    wq = nc.declare_dram_parameter("wq", [E, 128], F32R, isOutput=False)
    wk = nc.declare_dram_parameter("wk", [E, 128], F32R, isOutput=False)
    wv = nc.declare_dram_parameter("wv", [E, 128], F32R, isOutput=False)
    bqp = nc.declare_dram_parameter("bq", [128, 1], F32, isOutput=False)
    bkp = nc.declare_dram_parameter("bk", [128, 1], F32, isOutput=False)
    bvp = nc.declare_dram_parameter("bv", [128, 1], F32, isOutput=False)
    wo = nc.declare_dram_parameter("wo", [E, E], F32R, isOutput=False)
    bop = nc.declare_dram_parameter("bo", [8, 128], F32, isOutput=False)
    posf = nc.declare_dram_parameter("posf", [1, NT], F32, isOutput=False)
    thetap = nc.declare_dram_parameter("theta", [128, 1], F32, isOutput=False)
    identp = nc.declare_dram_parameter("ident", [128, 128], F32,
                                       isOutput=False)
    outp = nc.declare_dram_parameter("out", [E, NT // NCORES], F32,
                                     isOutput=True)

    # per-head-half AllToAll payloads (bf16): block g carries this core's
    # head (A|B) ctx dims for global tokens 512g..512g+512
    sendA = nc.dram_tensor("sendA", [NCORES, 64, 512], BF16)
    recvA = nc.dram_tensor("recvA", [NCORES, 64, 512], BF16)
    sendB = nc.dram_tensor("sendB", [NCORES, 64, 512], BF16)
    recvB = nc.dram_tensor("recvB", [NCORES, 64, 512], BF16)

    with tile.TileContext(nc) as tc:
        with tc.tile_pool(name="const", bufs=1) as cst, \
             tc.tile_pool(name="qmkm", bufs=1) as qmkm, \
             tc.tile_pool(name="vnat", bufs=1) as vnp, \
             tc.tile_pool(name="wop", bufs=1) as wop:
            th = cst.tile([128, 1], F32)
            nc.sync.dma_start(th[:], thetap[:])
            bq_t = cst.tile([128, 1], F32)
            nc.sync.dma_start(bq_t[:], bqp[:])
            bk_t = cst.tile([128, 1], F32)
            nc.sync.dma_start(bk_t[:], bkp[:])
            bv_t = cst.tile([128, 1], F32)
            nc.sync.dma_start(bv_t[:], bvp[:])
            ident = cst.tile([128, 128], F32)
            nc.sync.dma_start(ident[:], identp[:])
            onecol = cst.tile([128, 1], F32)
            nc.vector.memset(onecol[:], 1.0)

            Qm = qmkm.tile([128, NT], BF16)
            Km = qmkm.tile([128, NT], BF16)
            # V token-major with a ones column per head:
            # 32 token-blocks x (64 headA | 1 | 64 headB | 1) columns
            Vna = vnp.tile([128, 32 * 130], BF16)
            vna_v = Vna[:].rearrange("p (g h d) -> p g h d", g=32, h=2)
            nc.vector.tensor_copy(
                vna_v[:, :, :, 64:65],
                onecol[:, 0:1].unsqueeze(1).unsqueeze(1)
                .broadcast_to([128, 32, 2, 1]))

            if True:
                with tc.tile_pool(name="qk01", bufs=1) as qkp, \
                     tc.tile_pool(name="trigp", bufs=1) as trg, \
                     tc.tile_pool(name="wts", bufs=1) as wtp, \
                     tc.tile_pool(name="xr", bufs=2) as xrp, \
                     tc.tile_pool(name="vt", bufs=1) as vtp, \
                     tc.tile_pool(name="ps_proj", bufs=2, space="PSUM") as psp, \
                     tc.tile_pool(name="ps_vt", bufs=2, space="PSUM") as pvt:
                    # rows 0:64 = Q {Ax0,Bx0}/{Ax1,Bx1}, rows 64:128 = K
                    QK0 = qkp.tile([128, NT], F32)
                    QK1 = qkp.tile([128, NT], F32)
                    ones_r = trg.tile([1, 128], F32)
                    nc.vector.memset(ones_r[:], 1.0)
                    w_tiles = {}
                    for name, wsrc in (("q", wq), ("k", wk), ("v", wv)):
                        wr = wtp.tile([128, 8, 128], F32R, tag=f"w{name}")
                        nc.sync.dma_start(
                            wr[:], wsrc[:].rearrange("(a p) d -> p a d",
                                                     p=128))
                        w_tiles[name] = wr
                    VT = vtp.tile([128, NT], F32)
                    for t in range(8):
                        xr = xrp.tile([128, 8, 512], F32R, tag="xr")
                        nc.sync.dma_start(
                            xr[:],
                            xT[:, 512 * t:512 * (t + 1)].rearrange(
                                "(a p) n -> p a n", p=128))
                        cols = slice(512 * t, 512 * (t + 1))
                        for name, bias in (("q", bq_t), ("k", bk_t),
                                           ("v", bv_t)):
                            acc = psp.tile([128, 512], F32, tag="proj")
                            for e in range(8):
                                nc.tensor.matmul(
                                    acc[:], w_tiles[name][:, e, :],
                                    xr[:, e, :],
                                    start=(e == 0), stop=(e == 7))
                            if name == "v":
                                nc.scalar.activation(
                                    VT[:, cols], acc[:], AF.Identity,
                                    bias=bias[:])
                            else:
                                ro = 0 if name == "q" else 64
                                nc.scalar.activation(
                                    QK0[ro:ro + 64, cols], acc[0:64, :],
                                    AF.Identity, bias=bias[0:64, :])
                                nc.scalar.activation(
                                    QK1[ro:ro + 64, cols], acc[64:128, :],
                                    AF.Identity, bias=bias[64:128, :])
                        if t not in (3, 7):
                            continue
                        # half of the tokens is ready: V transpose + RoPE
                        # for these columns while the other half projects
                        hb = 0 if t == 3 else 1
                        hc = slice(2048 * hb, 2048 * (hb + 1))
                        # trig for this half: ang = pos*theta, Cody-Waite
                        # range-reduce, Sin via ACT spline
                        cos_t = trg.tile([128, 2048], F32, tag="cosh",
                                         name=f"cos{hb}")
                        sin_t = trg.tile([128, 2048], F32, tag="sinh",
                                         name=f"sin{hb}")
                        ang = trg.tile([128, 2048], F32, tag="tang",
                                       name=f"ang{hb}")
                        k_t = trg.tile([128, 2048], F32, tag="tk",
                                       name=f"tk{hb}")
                        t1 = trg.tile([128, 2048], F32, tag="tt1",
                                      name=f"tt1{hb}")
                        red = trg.tile([128, 2048], F32, tag="tred",
                                       name=f"tred{hb}")
                        pos_sb = trg.tile([1, 2048], F32, tag="pos",
                                          name=f"pos{hb}")
                        nc.sync.dma_start(
                            pos_sb[:], posf[:, 2048 * hb:2048 * (hb + 1)])
                        with tc.tile_pool(name=f"ps_ang{hb}", bufs=1,
                                          space="PSUM") as psa:
                            pb = psa.tile([128, 2048], F32, tag="angp",
                                          name=f"angp{hb}")
                            for j in range(4):
                                nc.tensor.matmul(
                                    pb[:, 512 * j:512 * (j + 1)], ones_r[:],
                                    pos_sb[:, 512 * j:512 * (j + 1)],
                                    start=True, stop=True)
                            nc.vector.tensor_scalar_mul(ang[:], pb[:], th[:])
                        nc.vector.tensor_scalar(
                            k_t[:], ang[:], INV2PI, MAGIC, A.mult, A.add)
                        nc.vector.tensor_scalar_sub(k_t[:], k_t[:], MAGIC)
                        nc.vector.scalar_tensor_tensor(
                            t1[:], k_t[:], -C1, ang[:], A.mult, A.add)
                        nc.vector.scalar_tensor_tensor(
                            red[:], k_t[:], -C2, t1[:], A.mult, A.add)
                        nc.scalar.activation(sin_t[:], red[:], AF.Sin)
                        nc.vector.tensor_scalar_add(t1[:], red[:], HALF_PI)
                        nc.vector.tensor_scalar(k_t[:], t1[:], PI, None,
                                                A.is_gt)
                        nc.vector.scalar_tensor_tensor(
                            ang[:], k_t[:], -TWO_PI, t1[:], A.mult, A.add)
                        nc.scalar.activation(cos_t[:], ang[:], AF.Sin)
                        for g in range(4 * hb, 4 * hb + 4):
                            ptile = pvt.tile([128, 512], F32, tag="vtp")
                            for j in range(4):
                                kb = 4 * g + j
                                nc.tensor.transpose(
                                    ptile[:, 128 * j:128 * (j + 1)],
                                    VT[:, 128 * kb:128 * (kb + 1)],
                                    ident[:])
                            src = ptile[:].rearrange(
                                "p (j h d) -> p j h d", j=4, h=2)
                            nc.vector.tensor_copy(
                                vna_v[:, 4 * g:4 * (g + 1), :, 0:64], src)
                        # RoPE in place: r0 -> QK0, r1 -> QK1
                        sA = trg.tile([128, 2048], F32, tag="tang",
                                      name=f"ra{hb}")
                        sB = trg.tile([128, 2048], F32, tag="tk",
                                      name=f"rb{hb}")
                        sC = trg.tile([128, 2048], F32, tag="tt1",
                                      name=f"rc{hb}")
                        nc.vector.tensor_mul(sA[:], QK0[:, hc], sin_t[:])
                        nc.vector.tensor_mul(sB[:], QK0[:, hc], cos_t[:])
                        nc.vector.tensor_mul(sC[:], QK1[:, hc], sin_t[:])
                        nc.vector.tensor_sub(QK0[:, hc], sB[:], sC[:])
                        nc.vector.tensor_mul(sB[:], QK1[:, hc], cos_t[:])
                        nc.vector.tensor_add(QK1[:, hc], sA[:], sB[:])
                        # merge to head-contiguous layout + bf16 round
                        for dst, ro in ((Qm, 0), (Km, 64)):
                            nc.vector.tensor_copy(dst[0:32, hc],
                                                  QK0[ro:ro + 32, hc])
                            nc.vector.tensor_copy(dst[32:64, hc],
                                                  QK1[ro:ro + 32, hc])
                            nc.vector.tensor_copy(dst[64:96, hc],
                                                  QK0[ro + 32:ro + 64, hc])
                            nc.vector.tensor_copy(dst[96:128, hc],
                                                  QK1[ro + 32:ro + 64, hc])

            # ---- attention ----
            # output projection weights stream in during attention
            wo_r = wop.tile([128, 8, 1024], F32R)
            nc.sync.dma_start(wo_r[:],
                              wo[:].rearrange("(a p) d -> p a d", p=128))
            bo_t = wop.tile([128, 8], F32)
            nc.sync.dma_start(bo_t[:], bop[:].rearrange("e p -> p e"))
            with tc.tile_pool(name="ctxu", bufs=1) as cxp, \
                 tc.tile_pool(name="nrm", bufs=1) as nrm:
                ctxu = [cxp.tile([65, 2048], F32, name=f"ctxu{p}",
                                 tag=f"cx{p}") for p in range(4)]
                ctxb = [nrm.tile([64, NT], BF16, name=f"ctxb{h}",
                                 tag=f"cb{h}") for h in range(2)]
                ones_row = nrm.tile([1, 64], F32)
                nc.vector.memset(ones_row[:], 1.0)
                ones_row_r = nrm.tile([1, 64], F32R)
                nc.vector.tensor_copy(ones_row_r[:], ones_row[:])
                sums_t = [nrm.tile([1, NT], F32, name=f"sums{h}",
                                   tag="sums") for h in range(2)]
                recip_t = [nrm.tile([1, NT], F32R, name=f"recip{h}",
                                    tag="recip") for h in range(2)]
                rep_t = [nrm.tile([64, NT], F32, name=f"rep{h}",
                                  tag="rep") for h in range(2)]
                with tc.tile_pool(name="pT", bufs=4) as ptp, \
                     tc.tile_pool(name="ps_sc", bufs=2, space="PSUM") as pssc, \
                     tc.tile_pool(name="ps_ctx", bufs=1, space="PSUM") as pscx:
                    for p, (b, h) in enumerate(PAIR_BH):
                        base = 2048 * b
                        hr = 64 * h
                        ctx_acc = [pscx.tile([65, 512], F32,
                                             name=f"ctxacc{p}_{q}",
                                             tag=f"ca{q}")
                                   for q in range(4)]
                        def emit_pv(pend):
                            pkb, phalf, ppT = pend
                            pvb = 16 * b + pkb
                            for qq in range(2):
                                q = 2 * phalf + qq
                                nc.tensor.matmul(
                                    ctx_acc[q][:, :],
                                    Vna[:, 130 * pvb + 65 * h:
                                        130 * pvb + 65 * (h + 1)],
                                    ppT[:, 512 * qq:512 * (qq + 1)],
                                    start=(pkb == 0), stop=(pkb == 15))
                        pending = None
                        for kb in range(16):
                            kcol = base + 128 * kb
                            for half in range(2):
                                sc = pssc.tile([128, 1024], F32, tag="sc",
                                               name=f"sc{p}_{kb}_{half}")
                                for qq in range(2):
                                    q = 2 * half + qq
                                    nc.tensor.matmul(
                                        sc[:, 512 * qq:512 * (qq + 1)],
                                        Km[hr:hr + 64, kcol:kcol + 128],
                                        Qm[hr:hr + 64,
                                           base + 512 * q:
                                           base + 512 * (q + 1)],
                                        start=True, stop=True)
                                pT = ptp.tile([128, 1024], BF16, tag="pT",
                                              name=f"pT{p}_{kb}_{half}")
                                nc.scalar.activation(pT[:], sc[:], AF.Exp,
                                                     scale=0.125)
                                if pending is not None:
                                    emit_pv(pending)
                                pending = (kb, half, pT)
                        emit_pv(pending)
                        for q in range(4):
                            nc.vector.tensor_copy(
                                ctxu[p][:, 512 * q:512 * (q + 1)],
                                ctx_acc[q][:])
                        nc.vector.tensor_copy(
                            sums_t[h][:, base:base + 2048],
                            ctxu[p][64:65, :])
                        if p % 2 != 1:
                            continue
                        # both batches of head h done: normalize + send
                        lns = nrm.tile([1, NT], F32, name=f"lns{h}",
                                       tag="lns")
                        nc.scalar.activation(lns[:], sums_t[h][:], AF.Ln)
                        nc.scalar.activation(recip_t[h][:], lns[:], AF.Exp,
                                             scale=-1.0)
                        for g in range(8):
                            # borrow a ctx_acc PSUM slot between pairs
                            rp_ = pscx.tile([64, 512], F32, tag=f"ca{g % 4}",
                                            name=f"rp{h}_{g}")
                            nc.tensor.matmul(
                                rp_[:], ones_row_r[:],
                                recip_t[h][:, 512 * g:512 * (g + 1)],
                                start=True, stop=True)
                            nc.vector.tensor_copy(
                                rep_t[h][:, 512 * g:512 * (g + 1)],
                                rp_[:])
                        # pairs for head h are p-1 (b=0) and p (b=1)
                        for pi, bb in ((p - 1, 0), (p, 1)):
                            nc.vector.tensor_mul(
                                ctxb[h][:, 2048 * bb:2048 * (bb + 1)],
                                ctxu[pi][0:64, :],
                                rep_t[h][:, 2048 * bb:2048 * (bb + 1)])
                        send = sendA if h == 0 else sendB
                        for g in range(8):
                            nc.sync.dma_start(
                                send[g], ctxb[h][:, 512 * g:512 * (g + 1)])
                        nc.gpsimd.collective_compute(
                            "AllToAll", A.bypass,
                            replica_groups=[list(range(NCORES))],
                            ins=[send[:].opt()],
                            outs=[(recvA if h == 0 else recvB)[:].opt()])

            # ---- output projection for this core's 512-token block ----
            # head-A contribution right after the first AllToAll so the PE
            # works while the second AllToAll is in flight
            with tc.tile_pool(name="wrhs", bufs=1) as wrp, \
                 tc.tile_pool(name="ps_o", bufs=1, space="PSUM") as pso:
                rhs_b = wrp.tile([128, 8, 512], BF16)
                rhs_r = wrp.tile([128, 8, 512], F32R)
                outsb = wrp.tile([128, 8, 512], F32)
                po = [pso.tile([128, 512], F32, tag=f"po{eo}",
                               name=f"po{eo}") for eo in range(8)]
                for e in range(8):
                    nc.sync.dma_start(rhs_b[0:64, e, :], recvA[e])
                nc.vector.tensor_copy(rhs_r[0:64, :, :], rhs_b[0:64, :, :])
                for eo in range(8):
                    for e in range(8):
                        nc.tensor.matmul(
                            po[eo][:], wo_r[0:64, e, 128 * eo:128 * (eo + 1)],
                            rhs_r[0:64, e, :],
                            start=(e == 0), stop=False)
                for e in range(8):
                    nc.sync.dma_start(rhs_b[64:128, e, :], recvB[e])
                nc.vector.tensor_copy(rhs_r[64:128, :, :],
                                      rhs_b[64:128, :, :])
                for eo in range(8):
                    for e in range(8):
                        nc.tensor.matmul(
                            po[eo][:],
                            wo_r[64:128, e, 128 * eo:128 * (eo + 1)],
                            rhs_r[64:128, e, :],
                            start=False, stop=(e == 7))
                    nc.scalar.activation(outsb[:, eo, :], po[eo][:],
                                         AF.Identity,
                                         bias=bo_t[:, eo:eo + 1])
                    nc.sync.dma_start(outp[128 * eo:128 * (eo + 1), :],
                                      outsb[:, eo, :])

    nc.finalize()
    _split_multisync(nc)
    return nc


_NC_CACHE = {}


def _get_nc(debug=False):
    if debug not in _NC_CACHE:
        _NC_CACHE[debug] = _build_nc()
    return _NC_CACHE[debug]


def _make_in_maps(x, positions, Wq, bq, Wk, bk, Wv, bv, Wo, bo):
    x = np.ascontiguousarray(np.asarray(x, dtype=np.float32))
    positions = np.asarray(positions)
    xT = np.ascontiguousarray(x.reshape(NT, E).T)            # [E, NT]
    posf = np.ascontiguousarray(
        positions.astype(np.float32).reshape(1, NT))
    i = np.arange(D // 2)
    theta32 = (10000.0 ** (-2.0 * i / D)).astype(np.float32)
    theta = np.ascontiguousarray(np.tile(theta32, 4).reshape(128, 1))
    ident = np.eye(128, dtype=np.float32)
    Wo_c = np.ascontiguousarray(np.asarray(Wo, dtype=np.float32))
    bo_c = np.ascontiguousarray(
        np.asarray(bo, dtype=np.float32).reshape(8, 128))

    in_maps = []
    ar32 = np.arange(32)
    for c in range(NCORES):
        hA, hB = 2 * c, 2 * c + 1
        perm = np.concatenate([
            64 * hA + 2 * ar32, 64 * hB + 2 * ar32,
            64 * hA + 2 * ar32 + 1, 64 * hB + 2 * ar32 + 1])
        vcols = np.concatenate([64 * hA + np.arange(64),
                                64 * hB + np.arange(64)])
        m = {
            "xT": xT,
            "posf": posf,
            "theta": theta,
            "ident": ident,
            "wq": np.ascontiguousarray(np.asarray(Wq, np.float32)[:, perm]),
            "wk": np.ascontiguousarray(np.asarray(Wk, np.float32)[:, perm]),
            "wv": np.ascontiguousarray(np.asarray(Wv, np.float32)[:, vcols]),
            "bq": np.ascontiguousarray(
                np.asarray(bq, np.float32)[perm].reshape(128, 1)),
            "bk": np.ascontiguousarray(
                np.asarray(bk, np.float32)[perm].reshape(128, 1)),
            "bv": np.ascontiguousarray(
                np.asarray(bv, np.float32)[vcols].reshape(128, 1)),
            "wo": Wo_c,
            "bo": bo_c,
        }
        in_maps.append(m)
    return in_maps


def kernel(x, positions, Wq, bq, Wk, bk, Wv, bv, Wo, bo,
           _trace=False, _tmpdir=None):
    nc = _get_nc()
    in_maps = _make_in_maps(x, positions, Wq, bq, Wk, bk, Wv, bv, Wo, bo)
    res = run_bass_kernel_spmd(nc, in_maps, list(range(NCORES)),
                               trace=_trace, tmpdir=_tmpdir)
    full_T = np.empty((E, NT), np.float32)
    for c in range(NCORES):
        full_T[:, 512 * c:512 * (c + 1)] = res.results[c]["out"]
    out = full_T.T.reshape(B, S, E).copy()
    if _trace:
        kernel._last_result = res
    return out


# revision 22
# speedup vs baseline: 1.0116x; 1.0116x over previous
"""Multi-head self-attention with RoPE on 8 Trainium2 NeuronCores.

Sharding: tensor-parallel over the 16 heads (2 heads per core) for the
QKV projections + attention, then an AllToAll that re-shards by token so
each core runs the output projection for its 512-token block.

QKV/Wo projections run as float32r (full-rate fp32 on the PE array,
~1e-4 rel); the attention matmuls (QK^T and PV) run in bf16 so the PE
array stays dense (fast FWL weight loads) and hot. Softmax skips the
max-subtraction (scores/8 stay in [-8, 8] for inputs with unit-variance
activations and 1/sqrt(E)-scaled weights) and gets its denominators for
free from an appended ones-row in the PV matmul. RoPE cos/sin come from
the integer positions with a Cody-Waite range reduction + the ACT
engine's Sin spline.
"""

import sys

for _p in ("/opt/trn_rl_repo", "/opt/pypackages"):
    if _p not in sys.path:
        sys.path.append(_p)

import numpy as np

import concourse.bass as bass
import concourse.mybir as mybir
import concourse.tile as tile
from concourse.bass_utils import run_bass_kernel_spmd
import bass_rust

A = mybir.AluOpType
F32 = mybir.dt.float32
F32R = mybir.dt.float32r
BF16 = mybir.dt.bfloat16
AF = mybir.ActivationFunctionType

B, S, E, H, D = 2, 2048, 1024, 16, 64
NT = B * S            # 4096 tokens, batch-major
NCORES = 8

TWO_PI = 2 * np.pi
INV2PI = float(np.float32(1.0 / TWO_PI))
MAGIC = 12582912.0    # 1.5 * 2^23: add+sub rounds fp32 to nearest int
C1 = 6.28125          # 2*pi split: C1 exact in fp32 with short mantissa
C2 = float(np.float32(TWO_PI - C1))
PI = float(np.pi)
HALF_PI = float(np.pi / 2)

# pair order: head-major so each head's two batches finish together
PAIR_BH = [(0, 0), (1, 0), (0, 1), (1, 1)]


def _split_multisync(nc, max_waits=1, max_updates=1):
    """This container's walrus accepts at most one sync-wait and one
    sync-update per instruction; split extras onto adjacent NoOps."""
    ctr = 0
    for f in nc.m.functions:
        for bb in f.blocks:
            new_list = []
            changed = False
            for ins in bb.instructions:
                si = ins.sync_info
                pre, post = [], []
                if si is not None:
                    waits = list(si.on_wait) if si.on_wait else []
                    if len(waits) > max_waits:
                        for w in waits[:-max_waits]:
                            ctr += 1
                            nop = bass_rust.InstNoOp(
                                name=f"I-mws-{ctr}", ins=[], outs=[])
                            nop.engine = ins.engine
                            nop.sync_info = bass_rust.SyncInfo(
                                on_wait=[w], on_update=[])
                            pre.append(nop)
                        si.on_wait = waits[-max_waits:]
                    upds = list(si.on_update) if si.on_update else []
                    if len(upds) > max_updates:
                        si.on_update = upds[:max_updates]
                        for u in upds[max_updates:]:
                            ctr += 1
                            nop = bass_rust.InstNoOp(
                                name=f"I-mus-{ctr}", ins=[], outs=[])
                            nop.engine = ins.engine
                            nop.sync_info = bass_rust.SyncInfo(
                                on_wait=[], on_update=[u])
                            post.append(nop)
                if pre or post:
                    changed = True
                new_list.extend(pre)
                new_list.append(ins)
                new_list.extend(post)
            if changed:
                bb.instructions = new_list


def _build_nc():
    nc = bass.Bass()

    xT = nc.declare_dram_parameter("xT", [E, NT], F32R, isOutput=False)
    wq = nc.declare_dram_parameter("wq", [E, 128], F32R, isOutput=False)
    wk = nc.declare_dram_parameter("wk", [E, 128], F32R, isOutput=False)
    wv = nc.declare_dram_parameter("wv", [E, 128], F32R, isOutput=False)
    bqp = nc.declare_dram_parameter("bq", [128, 1], F32, isOutput=False)
    bkp = nc.declare_dram_parameter("bk", [128, 1], F32, isOutput=False)
    bvp = nc.declare_dram_parameter("bv", [128, 1], F32, isOutput=False)
    wo = nc.declare_dram_parameter("wo", [E, E], F32R, isOutput=False)
    bop = nc.declare_dram_parameter("bo", [8, 128], F32, isOutput=False)
    posf = nc.declare_dram_parameter("posf", [1, NT], F32, isOutput=False)
    thetap = nc.declare_dram_parameter("theta", [128, 1], F32, isOutput=False)
    identp = nc.declare_dram_parameter("ident", [128, 128], F32,
                                       isOutput=False)
    outp = nc.declare_dram_parameter("out", [E, NT // NCORES], F32,
                                     isOutput=True)

    # per-head-half AllToAll payloads (bf16): block g carries this core's
    # head (A|B) ctx dims for global tokens 512g..512g+512
    sendA = nc.dram_tensor("sendA", [NCORES, 64, 512], BF16)
    recvA = nc.dram_tensor("recvA", [NCORES, 64, 512], BF16)
    sendB = nc.dram_tensor("sendB", [NCORES, 64, 512], BF16)
    recvB = nc.dram_tensor("recvB", [NCORES, 64, 512], BF16)

    with tile.TileContext(nc) as tc:
        with tc.tile_pool(name="const", bufs=1) as cst, \
             tc.tile_pool(name="qmkm", bufs=1) as qmkm, \
             tc.tile_pool(name="vnat", bufs=1) as vnp, \
             tc.tile_pool(name="wop", bufs=1) as wop:
            th = cst.tile([128, 1], F32)
            nc.sync.dma_start(th[:], thetap[:])
            bq_t = cst.tile([128, 1], F32)
            nc.sync.dma_start(bq_t[:], bqp[:])
            bk_t = cst.tile([128, 1], F32)
            nc.sync.dma_start(bk_t[:], bkp[:])
            bv_t = cst.tile([128, 1], F32)
            nc.sync.dma_start(bv_t[:], bvp[:])
            ident = cst.tile([128, 128], F32)
            nc.sync.dma_start(ident[:], identp[:])
            onecol = cst.tile([128, 1], F32)
            nc.vector.memset(onecol[:], 1.0)

            Qm = qmkm.tile([128, NT], BF16)
            Km = qmkm.tile([128, NT], BF16)
            # V token-major with a ones column per head:
            # 32 token-blocks x (64 headA | 1 | 64 headB | 1) columns
            Vna = vnp.tile([128, 32 * 130], BF16)
            vna_v = Vna[:].rearrange("p (g h d) -> p g h d", g=32, h=2)
            nc.vector.tensor_copy(
                vna_v[:, :, :, 64:65],
                onecol[:, 0:1].unsqueeze(1).unsqueeze(1)
                .broadcast_to([128, 32, 2, 1]))

            if True:
                with tc.tile_pool(name="qk01", bufs=1) as qkp, \
                     tc.tile_pool(name="trigp", bufs=1) as trg, \
                     tc.tile_pool(name="wts", bufs=1) as wtp, \
                     tc.tile_pool(name="xr", bufs=2) as xrp, \
                     tc.tile_pool(name="vt", bufs=1) as vtp, \
                     tc.tile_pool(name="ps_proj", bufs=2, space="PSUM") as psp, \
                     tc.tile_pool(name="ps_vt", bufs=2, space="PSUM") as pvt:
                    # rows 0:64 = Q {Ax0,Bx0}/{Ax1,Bx1}, rows 64:128 = K
                    QK0 = qkp.tile([128, NT], F32)
                    QK1 = qkp.tile([128, NT], F32)
                    ones_r = trg.tile([1, 128], F32)
                    nc.vector.memset(ones_r[:], 1.0)
                    w_tiles = {}
                    for name, wsrc in (("q", wq), ("k", wk), ("v", wv)):
                        wr = wtp.tile([128, 8, 128], F32R, tag=f"w{name}")
                        nc.sync.dma_start(
                            wr[:], wsrc[:].rearrange("(a p) d -> p a d",
                                                     p=128))
                        w_tiles[name] = wr
                    VT = vtp.tile([128, NT], F32)
                    for t in range(8):
                        xr = xrp.tile([128, 8, 512], F32R, tag="xr")
                        nc.sync.dma_start(
                            xr[:],
                            xT[:, 512 * t:512 * (t + 1)].rearrange(
                                "(a p) n -> p a n", p=128))
                        cols = slice(512 * t, 512 * (t + 1))
                        for name, bias in (("q", bq_t), ("k", bk_t),
                                           ("v", bv_t)):
                            acc = psp.tile([128, 512], F32, tag="proj")
                            for e in range(8):
                                nc.tensor.matmul(
                                    acc[:], w_tiles[name][:, e, :],
                                    xr[:, e, :],
                                    start=(e == 0), stop=(e == 7))
                            if name == "v":
                                nc.scalar.activation(
                                    VT[:, cols], acc[:], AF.Identity,
                                    bias=bias[:])
                            else:
                                ro = 0 if name == "q" else 64
                                nc.scalar.activation(
                                    QK0[ro:ro + 64, cols], acc[0:64, :],
                                    AF.Identity, bias=bias[0:64, :])
                                nc.scalar.activation(
                                    QK1[ro:ro + 64, cols], acc[64:128, :],
                                    AF.Identity, bias=bias[64:128, :])
                        if t not in (3, 7):
                            continue
                        # half of the tokens is ready: V transpose + RoPE
                        # for these columns while the other half projects
                        hb = 0 if t == 3 else 1
                        hc = slice(2048 * hb, 2048 * (hb + 1))
                        # trig for this half: ang = pos*theta, Cody-Waite
                        # range-reduce, Sin via ACT spline
                        cos_t = trg.tile([128, 2048], F32, tag="cosh",
                                         name=f"cos{hb}")
                        sin_t = trg.tile([128, 2048], F32, tag="sinh",
                                         name=f"sin{hb}")
                        ang = trg.tile([128, 2048], F32, tag="tang",
                                       name=f"ang{hb}")
                        k_t = trg.tile([128, 2048], F32, tag="tk",
                                       name=f"tk{hb}")
                        t1 = trg.tile([128, 2048], F32, tag="tt1",
                                      name=f"tt1{hb}")
                        red = trg.tile([128, 2048], F32, tag="tred",
                                       name=f"tred{hb}")
                        pos_sb = trg.tile([1, 2048], F32, tag="pos",
                                          name=f"pos{hb}")
                        nc.sync.dma_start(
                            pos_sb[:], posf[:, 2048 * hb:2048 * (hb + 1)])
                        with tc.tile_pool(name=f"ps_ang{hb}", bufs=1,
                                          space="PSUM") as psa:
                            pb = psa.tile([128, 2048], F32, tag="angp",
                                          name=f"angp{hb}")
                            for j in range(4):
                                nc.tensor.matmul(
                                    pb[:, 512 * j:512 * (j + 1)], ones_r[:],
                                    pos_sb[:, 512 * j:512 * (j + 1)],
                                    start=True, stop=True)
                            nc.vector.tensor_scalar_mul(ang[:], pb[:], th[:])
                        nc.vector.tensor_scalar(
                            k_t[:], ang[:], INV2PI, MAGIC, A.mult, A.add)
                        nc.vector.tensor_scalar_sub(k_t[:], k_t[:], MAGIC)
                        nc.vector.scalar_tensor_tensor(
                            t1[:], k_t[:], -C1, ang[:], A.mult, A.add)
                        nc.vector.scalar_tensor_tensor(
                            red[:], k_t[:], -C2, t1[:], A.mult, A.add)
                        nc.scalar.activation(sin_t[:], red[:], AF.Sin)
                        nc.vector.tensor_scalar_add(t1[:], red[:], HALF_PI)
                        nc.vector.tensor_scalar(k_t[:], t1[:], PI, None,
                                                A.is_gt)
                        nc.vector.scalar_tensor_tensor(
                            ang[:], k_t[:], -TWO_PI, t1[:], A.mult, A.add)
                        nc.scalar.activation(cos_t[:], ang[:], AF.Sin)
                        for g in range(4 * hb, 4 * hb + 4):
                            ptile = pvt.tile([128, 512], F32, tag="vtp")
                            for j in range(4):
                                kb = 4 * g + j
                                nc.tensor.transpose(
                                    ptile[:, 128 * j:128 * (j + 1)],
                                    VT[:, 128 * kb:128 * (kb + 1)],
                                    ident[:])
                            src = ptile[:].rearrange(
                                "p (j h d) -> p j h d", j=4, h=2)
                            nc.vector.tensor_copy(
                                vna_v[:, 4 * g:4 * (g + 1), :, 0:64], src)
                        # RoPE in place: r0 -> QK0, r1 -> QK1
                        sA = trg.tile([128, 2048], F32, tag="tang",
                                      name=f"ra{hb}")
                        sB = trg.tile([128, 2048], F32, tag="tk",
                                      name=f"rb{hb}")
                        sC = trg.tile([128, 2048], F32, tag="tt1",
                                      name=f"rc{hb}")
                        nc.vector.tensor_mul(sA[:], QK0[:, hc], sin_t[:])
                        nc.vector.tensor_mul(sB[:], QK0[:, hc], cos_t[:])
                        nc.vector.tensor_mul(sC[:], QK1[:, hc], sin_t[:])
                        nc.vector.tensor_sub(QK0[:, hc], sB[:], sC[:])
                        nc.vector.tensor_mul(sB[:], QK1[:, hc], cos_t[:])
                        nc.vector.tensor_add(QK1[:, hc], sA[:], sB[:])
                        # merge to head-contiguous layout + bf16 round
                        for dst, ro in ((Qm, 0), (Km, 64)):
                            nc.vector.tensor_copy(dst[0:32, hc],
                                                  QK0[ro:ro + 32, hc])
                            nc.vector.tensor_copy(dst[32:64, hc],
                                                  QK1[ro:ro + 32, hc])
                            nc.vector.tensor_copy(dst[64:96, hc],
                                                  QK0[ro + 32:ro + 64, hc])
                            nc.vector.tensor_copy(dst[96:128, hc],
                                                  QK1[ro + 32:ro + 64, hc])

            # ---- attention ----
            # output projection weights stream in during attention
            wo_r = wop.tile([128, 8, 1024], F32R)
            nc.sync.dma_start(wo_r[:],
                              wo[:].rearrange("(a p) d -> p a d", p=128))
            bo_t = wop.tile([128, 8], F32)
            nc.sync.dma_start(bo_t[:], bop[:].rearrange("e p -> p e"))
            with tc.tile_pool(name="ctxu", bufs=1) as cxp, \
                 tc.tile_pool(name="nrm", bufs=1) as nrm:
                ctxu = [cxp.tile([65, 2048], F32, name=f"ctxu{p}",
                                 tag=f"cx{p}") for p in range(4)]
                ctxb = [nrm.tile([64, NT], BF16, name=f"ctxb{h}",
                                 tag=f"cb{h}") for h in range(2)]
                ones_row = nrm.tile([1, 64], F32)
                nc.vector.memset(ones_row[:], 1.0)
                ones_row_r = nrm.tile([1, 64], F32R)
                nc.vector.tensor_copy(ones_row_r[:], ones_row[:])
                sums_t = [nrm.tile([1, NT], F32, name=f"sums{h}",
                                   tag="sums") for h in range(2)]
                recip_t = [nrm.tile([1, NT], F32R, name=f"recip{h}",
                                    tag="recip") for h in range(2)]
                rep_t = [nrm.tile([64, NT], F32, name=f"rep{h}",
                                  tag="rep") for h in range(2)]
                with tc.tile_pool(name="pT", bufs=4) as ptp, \
                     tc.tile_pool(name="ps_sc", bufs=2, space="PSUM") as pssc, \
                     tc.tile_pool(name="ps_ctx", bufs=1, space="PSUM") as pscx:
                    for p, (b, h) in enumerate(PAIR_BH):
                        base = 2048 * b
                        hr = 64 * h
                        ctx_acc = [pscx.tile([65, 512], F32,
                                             name=f"ctxacc{p}_{q}",
                                             tag=f"ca{q}")
                                   for q in range(4)]
                        def emit_pv(pend):
                            pkb, phalf, ppT = pend
                            pvb = 16 * b + pkb
                            for qq in range(2):
                                q = 2 * phalf + qq
                                nc.tensor.matmul(
                                    ctx_acc[q][:, :],
                                    Vna[:, 130 * pvb + 65 * h:
                                        130 * pvb + 65 * (h + 1)],
                                    ppT[:, 512 * qq:512 * (qq + 1)],
                                    start=(pkb == 0), stop=(pkb == 15))
                        for kb in range(16):
                            kcol = base + 128 * kb
                            for half in range(2):
                                sc = pssc.tile([128, 1024], F32, tag="sc",
                                               name=f"sc{p}_{kb}_{half}")
                                for qq in range(2):
                                    q = 2 * half + qq
                                    nc.tensor.matmul(
                                        sc[:, 512 * qq:512 * (qq + 1)],
                                        Km[hr:hr + 64, kcol:kcol + 128],
                                        Qm[hr:hr + 64,
                                           base + 512 * q:
                                           base + 512 * (q + 1)],
                                        start=True, stop=True)
                                pT = ptp.tile([128, 1024], BF16, tag="pT",
                                              name=f"pT{p}_{kb}_{half}")
                                nc.scalar.activation(pT[:], sc[:], AF.Exp,
                                                     scale=0.125)
                                emit_pv((kb, half, pT))
                        for q in range(4):
                            nc.vector.tensor_copy(
                                ctxu[p][:, 512 * q:512 * (q + 1)],
                                ctx_acc[q][:])
                        nc.vector.tensor_copy(
                            sums_t[h][:, base:base + 2048],
                            ctxu[p][64:65, :])
                        if p % 2 != 1:
                            continue
                        # both batches of head h done: normalize + send
                        lns = nrm.tile([1, NT], F32, name=f"lns{h}",
                                       tag="lns")
                        nc.scalar.activation(lns[:], sums_t[h][:], AF.Ln)
                        nc.scalar.activation(recip_t[h][:], lns[:], AF.Exp,
                                             scale=-1.0)
                        for g in range(8):
                            # borrow a ctx_acc PSUM slot between pairs
                            rp_ = pscx.tile([64, 512], F32, tag=f"ca{g % 4}",
                                            name=f"rp{h}_{g}")
                            nc.tensor.matmul(
                                rp_[:], ones_row_r[:],
                                recip_t[h][:, 512 * g:512 * (g + 1)],
                                start=True, stop=True)
                            nc.vector.tensor_copy(
                                rep_t[h][:, 512 * g:512 * (g + 1)],
                                rp_[:])
                        # pairs for head h are p-1 (b=0) and p (b=1)
                        for pi, bb in ((p - 1, 0), (p, 1)):
                            nc.vector.tensor_mul(
                                ctxb[h][:, 2048 * bb:2048 * (bb + 1)],
                                ctxu[pi][0:64, :],
                                rep_t[h][:, 2048 * bb:2048 * (bb + 1)])
                        send = sendA if h == 0 else sendB
                        for g in range(8):
                            nc.sync.dma_start(
                                send[g], ctxb[h][:, 512 * g:512 * (g + 1)])
                        nc.gpsimd.collective_compute(
                            "AllToAll", A.bypass,
                            replica_groups=[list(range(NCORES))],
                            ins=[send[:].opt()],
                            outs=[(recvA if h == 0 else recvB)[:].opt()])

            # ---- output projection for this core's 512-token block ----
            # head-A contribution right after the first AllToAll so the PE
            # works while the second AllToAll is in flight
            with tc.tile_pool(name="wrhs", bufs=1) as wrp, \
                 tc.tile_pool(name="ps_o", bufs=1, space="PSUM") as pso:
                rhs_b = wrp.tile([128, 8, 512], BF16)
                rhs_r = wrp.tile([128, 8, 512], F32R)
                outsb = wrp.tile([128, 8, 512], F32)
                po = [pso.tile([128, 512], F32, tag=f"po{eo}",
                               name=f"po{eo}") for eo in range(8)]
                for e in range(8):
                    nc.sync.dma_start(rhs_b[0:64, e, :], recvA[e])
                nc.vector.tensor_copy(rhs_r[0:64, :, :], rhs_b[0:64, :, :])
                for eo in range(8):
                    for e in range(8):
                        nc.tensor.matmul(
                            po[eo][:], wo_r[0:64, e, 128 * eo:128 * (eo + 1)],
                            rhs_r[0:64, e, :],
                            start=(e == 0), stop=False)
                for e in range(8):
                    nc.sync.dma_start(rhs_b[64:128, e, :], recvB[e])
                nc.vector.tensor_copy(rhs_r[64:128, :, :],
                                      rhs_b[64:128, :, :])
                for eo in range(8):
                    for e in range(8):
                        nc.tensor.matmul(
                            po[eo][:],
                            wo_r[64:128, e, 128 * eo:128 * (eo + 1)],
                            rhs_r[64:128, e, :],
                            start=False, stop=(e == 7))
                    nc.scalar.activation(outsb[:, eo, :], po[eo][:],
                                         AF.Identity,
                                         bias=bo_t[:, eo:eo + 1])
                    nc.sync.dma_start(outp[128 * eo:128 * (eo + 1), :],
                                      outsb[:, eo, :])

    nc.finalize()
    _split_multisync(nc)
    return nc


_NC_CACHE = {}


def _get_nc(debug=False):
    if debug not in _NC_CACHE:
        _NC_CACHE[debug] = _build_nc()
    return _NC_CACHE[debug]


def _make_in_maps(x, positions, Wq, bq, Wk, bk, Wv, bv, Wo, bo):
    x = np.ascontiguousarray(np.asarray(x, dtype=np.float32))
    positions = np.asarray(positions)
    xT = np.ascontiguousarray(x.reshape(NT, E).T)            # [E, NT]
    posf = np.ascontiguousarray(
        positions.astype(np.float32).reshape(1, NT))
    i = np.arange(D // 2)
    theta32 = (10000.0 ** (-2.0 * i / D)).astype(np.float32)
    theta = np.ascontiguousarray(np.tile(theta32, 4).reshape(128, 1))
    ident = np.eye(128, dtype=np.float32)
    Wo_c = np.ascontiguousarray(np.asarray(Wo, dtype=np.float32))
    bo_c = np.ascontiguousarray(
        np.asarray(bo, dtype=np.float32).reshape(8, 128))

    in_maps = []
    ar32 = np.arange(32)
    for c in range(NCORES):
        hA, hB = 2 * c, 2 * c + 1
        perm = np.concatenate([
            64 * hA + 2 * ar32, 64 * hB + 2 * ar32,
            64 * hA + 2 * ar32 + 1, 64 * hB + 2 * ar32 + 1])
        vcols = np.concatenate([64 * hA + np.arange(64),
                                64 * hB + np.arange(64)])
        m = {
            "xT": xT,
            "posf": posf,
            "theta": theta,
            "ident": ident,
            "wq": np.ascontiguousarray(np.asarray(Wq, np.float32)[:, perm]),
            "wk": np.ascontiguousarray(np.asarray(Wk, np.float32)[:, perm]),
            "wv": np.ascontiguousarray(np.asarray(Wv, np.float32)[:, vcols]),
            "bq": np.ascontiguousarray(
                np.asarray(bq, np.float32)[perm].reshape(128, 1)),
            "bk": np.ascontiguousarray(
                np.asarray(bk, np.float32)[perm].reshape(128, 1)),
            "bv": np.ascontiguousarray(
                np.asarray(bv, np.float32)[vcols].reshape(128, 1)),
            "wo": Wo_c,
            "bo": bo_c,
        }
        in_maps.append(m)
    return in_maps


def kernel(x, positions, Wq, bq, Wk, bk, Wv, bv, Wo, bo,
           _trace=False, _tmpdir=None):
    nc = _get_nc()
    in_maps = _make_in_maps(x, positions, Wq, bq, Wk, bk, Wv, bv, Wo, bo)
    res = run_bass_kernel_spmd(nc, in_maps, list(range(NCORES)),
                               trace=_trace, tmpdir=_tmpdir)
    full_T = np.empty((E, NT), np.float32)
    for c in range(NCORES):
        full_T[:, 512 * c:512 * (c + 1)] = res.results[c]["out"]
    out = full_T.T.reshape(B, S, E).copy()
    if _trace:
        kernel._last_result = res
    return out


# revision 23
# speedup vs baseline: 1.0536x; 1.0416x over previous
"""Multi-head self-attention with RoPE on 8 Trainium2 NeuronCores.

Sharding: tensor-parallel over the 16 heads (2 heads per core) for the
QKV projections + attention, then an AllToAll that re-shards by token so
each core runs the output projection for its 512-token block.

QKV/Wo projections run as float32r (full-rate fp32 on the PE array,
~1e-4 rel); the attention matmuls (QK^T and PV) run in bf16 so the PE
array stays dense (fast FWL weight loads) and hot. Softmax skips the
max-subtraction (scores/8 stay in [-8, 8] for inputs with unit-variance
activations and 1/sqrt(E)-scaled weights) and gets its denominators for
free from an appended ones-row in the PV matmul. RoPE cos/sin come from
the integer positions with a Cody-Waite range reduction + the ACT
engine's Sin spline.
"""

import sys

for _p in ("/opt/trn_rl_repo", "/opt/pypackages"):
    if _p not in sys.path:
        sys.path.append(_p)

import numpy as np

import concourse.bass as bass
import concourse.mybir as mybir
import concourse.tile as tile
from concourse.bass_utils import run_bass_kernel_spmd
import bass_rust

A = mybir.AluOpType
F32 = mybir.dt.float32
F32R = mybir.dt.float32r
BF16 = mybir.dt.bfloat16
AF = mybir.ActivationFunctionType

B, S, E, H, D = 2, 2048, 1024, 16, 64
NT = B * S            # 4096 tokens, batch-major
NCORES = 8

TWO_PI = 2 * np.pi
INV2PI = float(np.float32(1.0 / TWO_PI))
MAGIC = 12582912.0    # 1.5 * 2^23: add+sub rounds fp32 to nearest int
C1 = 6.28125          # 2*pi split: C1 exact in fp32 with short mantissa
C2 = float(np.float32(TWO_PI - C1))
PI = float(np.pi)
HALF_PI = float(np.pi / 2)

# pair order: head-major so each head's two batches finish together
PAIR_BH = [(0, 0), (1, 0), (0, 1), (1, 1)]


def _split_multisync(nc, max_waits=1, max_updates=1):
    """This container's walrus accepts at most one sync-wait and one
    sync-update per instruction; split extras onto adjacent NoOps."""
    ctr = 0
    for f in nc.m.functions:
        for bb in f.blocks:
            new_list = []
            changed = False
            for ins in bb.instructions:
                si = ins.sync_info
                pre, post = [], []
                if si is not None:
                    waits = list(si.on_wait) if si.on_wait else []
                    if len(waits) > max_waits:
                        for w in waits[:-max_waits]:
                            ctr += 1
                            nop = bass_rust.InstNoOp(
                                name=f"I-mws-{ctr}", ins=[], outs=[])
                            nop.engine = ins.engine
                            nop.sync_info = bass_rust.SyncInfo(
                                on_wait=[w], on_update=[])
                            pre.append(nop)
                        si.on_wait = waits[-max_waits:]
                    upds = list(si.on_update) if si.on_update else []
                    if len(upds) > max_updates:
                        si.on_update = upds[:max_updates]
                        for u in upds[max_updates:]:
                            ctr += 1
                            nop = bass_rust.InstNoOp(
                                name=f"I-mus-{ctr}", ins=[], outs=[])
                            nop.engine = ins.engine
                            nop.sync_info = bass_rust.SyncInfo(
                                on_wait=[], on_update=[u])
                            post.append(nop)
                if pre or post:
                    changed = True
                new_list.extend(pre)
                new_list.append(ins)
                new_list.extend(post)
            if changed:
                bb.instructions = new_list


def _build_nc():
    nc = bass.Bass()

    xT = nc.declare_dram_parameter("xT", [E, NT], BF16, isOutput=False)
    wq = nc.declare_dram_parameter("wq", [E, 128], BF16, isOutput=False)
    wk = nc.declare_dram_parameter("wk", [E, 128], BF16, isOutput=False)
    wv = nc.declare_dram_parameter("wv", [E, 128], BF16, isOutput=False)
    bqp = nc.declare_dram_parameter("bq", [128, 1], F32, isOutput=False)
    bkp = nc.declare_dram_parameter("bk", [128, 1], F32, isOutput=False)
    bvp = nc.declare_dram_parameter("bv", [128, 1], F32, isOutput=False)
    wo = nc.declare_dram_parameter("wo", [E, E], F32R, isOutput=False)
    bop = nc.declare_dram_parameter("bo", [8, 128], F32, isOutput=False)
    posf = nc.declare_dram_parameter("posf", [1, NT], F32, isOutput=False)
    thetap = nc.declare_dram_parameter("theta", [128, 1], F32, isOutput=False)
    identp = nc.declare_dram_parameter("ident", [128, 128], F32,
                                       isOutput=False)
    outp = nc.declare_dram_parameter("out", [E, NT // NCORES], F32,
                                     isOutput=True)

    # per-head-half AllToAll payloads (bf16): block g carries this core's
    # head (A|B) ctx dims for global tokens 512g..512g+512
    sendA = nc.dram_tensor("sendA", [NCORES, 64, 512], BF16)
    recvA = nc.dram_tensor("recvA", [NCORES, 64, 512], BF16)
    sendB = nc.dram_tensor("sendB", [NCORES, 64, 512], BF16)
    recvB = nc.dram_tensor("recvB", [NCORES, 64, 512], BF16)

    with tile.TileContext(nc) as tc:
        with tc.tile_pool(name="const", bufs=1) as cst, \
             tc.tile_pool(name="qmkm", bufs=1) as qmkm, \
             tc.tile_pool(name="vnat", bufs=1) as vnp, \
             tc.tile_pool(name="wop", bufs=1) as wop:
            th = cst.tile([128, 1], F32)
            nc.sync.dma_start(th[:], thetap[:])
            bq_t = cst.tile([128, 1], F32)
            nc.sync.dma_start(bq_t[:], bqp[:])
            bk_t = cst.tile([128, 1], F32)
            nc.sync.dma_start(bk_t[:], bkp[:])
            bv_t = cst.tile([128, 1], F32)
            nc.sync.dma_start(bv_t[:], bvp[:])
            ident = cst.tile([128, 128], F32)
            nc.sync.dma_start(ident[:], identp[:])
            onecol = cst.tile([128, 1], F32)
            nc.vector.memset(onecol[:], 1.0)

            Qm = qmkm.tile([128, NT], BF16)
            Km = qmkm.tile([128, NT], BF16)
            # V token-major with a ones column per head:
            # 32 token-blocks x (64 headA | 1 | 64 headB | 1) columns
            Vna = vnp.tile([128, 32 * 130], BF16)
            vna_v = Vna[:].rearrange("p (g h d) -> p g h d", g=32, h=2)
            nc.vector.tensor_copy(
                vna_v[:, :, :, 64:65],
                onecol[:, 0:1].unsqueeze(1).unsqueeze(1)
                .broadcast_to([128, 32, 2, 1]))

            if True:
                with tc.tile_pool(name="qk01", bufs=1) as qkp, \
                     tc.tile_pool(name="trigp", bufs=1) as trg, \
                     tc.tile_pool(name="wts", bufs=1) as wtp, \
                     tc.tile_pool(name="xr", bufs=2) as xrp, \
                     tc.tile_pool(name="vt", bufs=1) as vtp, \
                     tc.tile_pool(name="ps_proj", bufs=2, space="PSUM") as psp, \
                     tc.tile_pool(name="ps_vt", bufs=2, space="PSUM") as pvt:
                    # rows 0:64 = Q {Ax0,Bx0}/{Ax1,Bx1}, rows 64:128 = K
                    QK0 = qkp.tile([128, NT], F32)
                    QK1 = qkp.tile([128, NT], F32)
                    ones_r = trg.tile([1, 128], F32)
                    nc.vector.memset(ones_r[:], 1.0)
                    w_tiles = {}
                    for name, wsrc in (("q", wq), ("k", wk), ("v", wv)):
                        wr = wtp.tile([128, 8, 128], BF16, tag=f"w{name}")
                        nc.sync.dma_start(
                            wr[:], wsrc[:].rearrange("(a p) d -> p a d",
                                                     p=128))
                        w_tiles[name] = wr
                    VT = vtp.tile([128, NT], F32)
                    for t in range(8):
                        xr = xrp.tile([128, 8, 512], BF16, tag="xr")
                        nc.sync.dma_start(
                            xr[:],
                            xT[:, 512 * t:512 * (t + 1)].rearrange(
                                "(a p) n -> p a n", p=128))
                        cols = slice(512 * t, 512 * (t + 1))
                        for name, bias in (("q", bq_t), ("k", bk_t),
                                           ("v", bv_t)):
                            acc = psp.tile([128, 512], F32, tag="proj")
                            for e in range(8):
                                nc.tensor.matmul(
                                    acc[:], w_tiles[name][:, e, :],
                                    xr[:, e, :],
                                    start=(e == 0), stop=(e == 7))
                            if name == "v":
                                nc.scalar.activation(
                                    VT[:, cols], acc[:], AF.Identity,
                                    bias=bias[:])
                            else:
                                ro = 0 if name == "q" else 64
                                nc.scalar.activation(
                                    QK0[ro:ro + 64, cols], acc[0:64, :],
                                    AF.Identity, bias=bias[0:64, :])
                                nc.scalar.activation(
                                    QK1[ro:ro + 64, cols], acc[64:128, :],
                                    AF.Identity, bias=bias[64:128, :])
                        if t not in (3, 7):
                            continue
                        # half of the tokens is ready: V transpose + RoPE
                        # for these columns while the other half projects
                        hb = 0 if t == 3 else 1
                        hc = slice(2048 * hb, 2048 * (hb + 1))
                        # trig for this half: ang = pos*theta, Cody-Waite
                        # range-reduce, Sin via ACT spline
                        cos_t = trg.tile([128, 2048], F32, tag="cosh",
                                         name=f"cos{hb}")
                        sin_t = trg.tile([128, 2048], F32, tag="sinh",
                                         name=f"sin{hb}")
                        ang = trg.tile([128, 2048], F32, tag="tang",
                                       name=f"ang{hb}")
                        k_t = trg.tile([128, 2048], F32, tag="tk",
                                       name=f"tk{hb}")
                        t1 = trg.tile([128, 2048], F32, tag="tt1",
                                      name=f"tt1{hb}")
                        red = trg.tile([128, 2048], F32, tag="tred",
                                       name=f"tred{hb}")
                        pos_sb = trg.tile([1, 2048], F32, tag="pos",
                                          name=f"pos{hb}")
                        nc.sync.dma_start(
                            pos_sb[:], posf[:, 2048 * hb:2048 * (hb + 1)])
                        with tc.tile_pool(name=f"ps_ang{hb}", bufs=1,
                                          space="PSUM") as psa:
                            pb = psa.tile([128, 2048], F32, tag="angp",
                                          name=f"angp{hb}")
                            for j in range(4):
                                nc.tensor.matmul(
                                    pb[:, 512 * j:512 * (j + 1)], ones_r[:],
                                    pos_sb[:, 512 * j:512 * (j + 1)],
                                    start=True, stop=True)
                            nc.vector.tensor_scalar_mul(ang[:], pb[:], th[:])
                        nc.vector.tensor_scalar(
                            k_t[:], ang[:], INV2PI, MAGIC, A.mult, A.add)
                        nc.vector.tensor_scalar_sub(k_t[:], k_t[:], MAGIC)
                        nc.vector.scalar_tensor_tensor(
                            t1[:], k_t[:], -C1, ang[:], A.mult, A.add)
                        nc.vector.scalar_tensor_tensor(
                            red[:], k_t[:], -C2, t1[:], A.mult, A.add)
                        nc.scalar.activation(sin_t[:], red[:], AF.Sin)
                        nc.vector.tensor_scalar_add(t1[:], red[:], HALF_PI)
                        nc.vector.tensor_scalar(k_t[:], t1[:], PI, None,
                                                A.is_gt)
                        nc.vector.scalar_tensor_tensor(
                            ang[:], k_t[:], -TWO_PI, t1[:], A.mult, A.add)
                        nc.scalar.activation(cos_t[:], ang[:], AF.Sin)
                        for g in range(4 * hb, 4 * hb + 4):
                            ptile = pvt.tile([128, 512], F32, tag="vtp")
                            for j in range(4):
                                kb = 4 * g + j
                                nc.tensor.transpose(
                                    ptile[:, 128 * j:128 * (j + 1)],
                                    VT[:, 128 * kb:128 * (kb + 1)],
                                    ident[:])
                            src = ptile[:].rearrange(
                                "p (j h d) -> p j h d", j=4, h=2)
                            nc.vector.tensor_copy(
                                vna_v[:, 4 * g:4 * (g + 1), :, 0:64], src)
                        # RoPE in place: r0 -> QK0, r1 -> QK1
                        sA = trg.tile([128, 2048], F32, tag="tang",
                                      name=f"ra{hb}")
                        sB = trg.tile([128, 2048], F32, tag="tk",
                                      name=f"rb{hb}")
                        sC = trg.tile([128, 2048], F32, tag="tt1",
                                      name=f"rc{hb}")
                        nc.vector.tensor_mul(sA[:], QK0[:, hc], sin_t[:])
                        nc.vector.tensor_mul(sB[:], QK0[:, hc], cos_t[:])
                        nc.vector.tensor_mul(sC[:], QK1[:, hc], sin_t[:])
                        nc.vector.tensor_sub(QK0[:, hc], sB[:], sC[:])
                        nc.vector.tensor_mul(sB[:], QK1[:, hc], cos_t[:])
                        nc.vector.tensor_add(QK1[:, hc], sA[:], sB[:])
                        # merge to head-contiguous layout + bf16 round
                        for dst, ro in ((Qm, 0), (Km, 64)):
                            nc.vector.tensor_copy(dst[0:32, hc],
                                                  QK0[ro:ro + 32, hc])
                            nc.vector.tensor_copy(dst[32:64, hc],
                                                  QK1[ro:ro + 32, hc])
                            nc.vector.tensor_copy(dst[64:96, hc],
                                                  QK0[ro + 32:ro + 64, hc])
                            nc.vector.tensor_copy(dst[96:128, hc],
                                                  QK1[ro + 32:ro + 64, hc])

            # ---- attention ----
            # output projection weights stream in during attention
            wo_r = wop.tile([128, 8, 1024], F32R)
            nc.sync.dma_start(wo_r[:],
                              wo[:].rearrange("(a p) d -> p a d", p=128))
            bo_t = wop.tile([128, 8], F32)
            nc.sync.dma_start(bo_t[:], bop[:].rearrange("e p -> p e"))
            with tc.tile_pool(name="ctxu", bufs=1) as cxp, \
                 tc.tile_pool(name="nrm", bufs=1) as nrm:
                ctxu = [cxp.tile([65, 2048], F32, name=f"ctxu{p}",
                                 tag=f"cx{p}") for p in range(4)]
                ctxb = [nrm.tile([64, NT], BF16, name=f"ctxb{h}",
                                 tag=f"cb{h}") for h in range(2)]
                ones_row = nrm.tile([1, 64], F32)
                nc.vector.memset(ones_row[:], 1.0)
                ones_row_r = nrm.tile([1, 64], F32R)
                nc.vector.tensor_copy(ones_row_r[:], ones_row[:])
                sums_t = [nrm.tile([1, NT], F32, name=f"sums{h}",
                                   tag="sums") for h in range(2)]
                recip_t = [nrm.tile([1, NT], F32R, name=f"recip{h}",
                                    tag="recip") for h in range(2)]
                rep_t = [nrm.tile([64, NT], F32, name=f"rep{h}",
                                  tag="rep") for h in range(2)]
                with tc.tile_pool(name="pT", bufs=4) as ptp, \
                     tc.tile_pool(name="ps_sc", bufs=2, space="PSUM") as pssc, \
                     tc.tile_pool(name="ps_ctx", bufs=1, space="PSUM") as pscx:
                    for p, (b, h) in enumerate(PAIR_BH):
                        base = 2048 * b
                        hr = 64 * h
                        ctx_acc = [pscx.tile([65, 512], F32,
                                             name=f"ctxacc{p}_{q}",
                                             tag=f"ca{q}")
                                   for q in range(4)]
                        def emit_pv(pend):
                            pkb, phalf, ppT = pend
                            pvb = 16 * b + pkb
                            for qq in range(2):
                                q = 2 * phalf + qq
                                nc.tensor.matmul(
                                    ctx_acc[q][:, :],
                                    Vna[:, 130 * pvb + 65 * h:
                                        130 * pvb + 65 * (h + 1)],
                                    ppT[:, 512 * qq:512 * (qq + 1)],
                                    start=(pkb == 0), stop=(pkb == 15))
                        for kb in range(16):
                            kcol = base + 128 * kb
                            for half in range(2):
                                sc = pssc.tile([128, 1024], F32, tag="sc",
                                               name=f"sc{p}_{kb}_{half}")
                                for qq in range(2):
                                    q = 2 * half + qq
                                    nc.tensor.matmul(
                                        sc[:, 512 * qq:512 * (qq + 1)],
                                        Km[hr:hr + 64, kcol:kcol + 128],
                                        Qm[hr:hr + 64,
                                           base + 512 * q:
                                           base + 512 * (q + 1)],
                                        start=True, stop=True)
                                pT = ptp.tile([128, 1024], BF16, tag="pT",
                                              name=f"pT{p}_{kb}_{half}")
                                nc.scalar.activation(pT[:], sc[:], AF.Exp,
                                                     scale=0.125)
                                emit_pv((kb, half, pT))
                        for q in range(4):
                            nc.vector.tensor_copy(
                                ctxu[p][:, 512 * q:512 * (q + 1)],
                                ctx_acc[q][:])
                        nc.vector.tensor_copy(
                            sums_t[h][:, base:base + 2048],
                            ctxu[p][64:65, :])
                        if p % 2 != 1:
                            continue
                        # both batches of head h done: normalize + send
                        lns = nrm.tile([1, NT], F32, name=f"lns{h}",
                                       tag="lns")
                        nc.scalar.activation(lns[:], sums_t[h][:], AF.Ln)
                        nc.scalar.activation(recip_t[h][:], lns[:], AF.Exp,
                                             scale=-1.0)
                        for g in range(8):
                            # borrow a ctx_acc PSUM slot between pairs
                            rp_ = pscx.tile([64, 512], F32, tag=f"ca{g % 4}",
                                            name=f"rp{h}_{g}")
                            nc.tensor.matmul(
                                rp_[:], ones_row_r[:],
                                recip_t[h][:, 512 * g:512 * (g + 1)],
                                start=True, stop=True)
                            nc.vector.tensor_copy(
                                rep_t[h][:, 512 * g:512 * (g + 1)],
                                rp_[:])
                        # pairs for head h are p-1 (b=0) and p (b=1)
                        for pi, bb in ((p - 1, 0), (p, 1)):
                            nc.vector.tensor_mul(
                                ctxb[h][:, 2048 * bb:2048 * (bb + 1)],
                                ctxu[pi][0:64, :],
                                rep_t[h][:, 2048 * bb:2048 * (bb + 1)])
                        send = sendA if h == 0 else sendB
                        for g in range(8):
                            nc.sync.dma_start(
                                send[g], ctxb[h][:, 512 * g:512 * (g + 1)])
                        nc.gpsimd.collective_compute(
                            "AllToAll", A.bypass,
                            replica_groups=[list(range(NCORES))],
                            ins=[send[:].opt()],
                            outs=[(recvA if h == 0 else recvB)[:].opt()])

            # ---- output projection for this core's 512-token block ----
            # head-A contribution right after the first AllToAll so the PE
            # works while the second AllToAll is in flight
            with tc.tile_pool(name="wrhs", bufs=1) as wrp, \
                 tc.tile_pool(name="ps_o", bufs=1, space="PSUM") as pso:
                rhs_b = wrp.tile([128, 8, 512], BF16)
                rhs_r = wrp.tile([128, 8, 512], F32R)
                outsb = wrp.tile([128, 8, 512], F32)
                po = [pso.tile([128, 512], F32, tag=f"po{eo}",
                               name=f"po{eo}") for eo in range(8)]
                for e in range(8):
                    nc.sync.dma_start(rhs_b[0:64, e, :], recvA[e])
                nc.vector.tensor_copy(rhs_r[0:64, :, :], rhs_b[0:64, :, :])
                for eo in range(8):
                    for e in range(8):
                        nc.tensor.matmul(
                            po[eo][:], wo_r[0:64, e, 128 * eo:128 * (eo + 1)],
                            rhs_r[0:64, e, :],
                            start=(e == 0), stop=False)
                for e in range(8):
                    nc.sync.dma_start(rhs_b[64:128, e, :], recvB[e])
                nc.vector.tensor_copy(rhs_r[64:128, :, :],
                                      rhs_b[64:128, :, :])
                for eo in range(8):
                    for e in range(8):
                        nc.tensor.matmul(
                            po[eo][:],
                            wo_r[64:128, e, 128 * eo:128 * (eo + 1)],
                            rhs_r[64:128, e, :],
                            start=False, stop=(e == 7))
                    nc.scalar.activation(outsb[:, eo, :], po[eo][:],
                                         AF.Identity,
                                         bias=bo_t[:, eo:eo + 1])
                    nc.sync.dma_start(outp[128 * eo:128 * (eo + 1), :],
                                      outsb[:, eo, :])

    nc.finalize()
    _split_multisync(nc)
    return nc


_NC_CACHE = {}


def _get_nc(debug=False):
    if debug not in _NC_CACHE:
        _NC_CACHE[debug] = _build_nc()
    return _NC_CACHE[debug]


def _make_in_maps(x, positions, Wq, bq, Wk, bk, Wv, bv, Wo, bo):
    import ml_dtypes
    bf16 = ml_dtypes.bfloat16
    x = np.ascontiguousarray(np.asarray(x, dtype=np.float32))
    positions = np.asarray(positions)
    xT = np.ascontiguousarray(x.reshape(NT, E).T.astype(bf16))  # [E, NT]
    posf = np.ascontiguousarray(
        positions.astype(np.float32).reshape(1, NT))
    i = np.arange(D // 2)
    theta32 = (10000.0 ** (-2.0 * i / D)).astype(np.float32)
    theta = np.ascontiguousarray(np.tile(theta32, 4).reshape(128, 1))
    ident = np.eye(128, dtype=np.float32)
    Wo_c = np.ascontiguousarray(np.asarray(Wo, dtype=np.float32))
    bo_c = np.ascontiguousarray(
        np.asarray(bo, dtype=np.float32).reshape(8, 128))

    in_maps = []
    ar32 = np.arange(32)
    for c in range(NCORES):
        hA, hB = 2 * c, 2 * c + 1
        perm = np.concatenate([
            64 * hA + 2 * ar32, 64 * hB + 2 * ar32,
            64 * hA + 2 * ar32 + 1, 64 * hB + 2 * ar32 + 1])
        vcols = np.concatenate([64 * hA + np.arange(64),
                                64 * hB + np.arange(64)])
        m = {
            "xT": xT,
            "posf": posf,
            "theta": theta,
            "ident": ident,
            "wq": np.ascontiguousarray(np.asarray(Wq, np.float32)[:, perm].astype(bf16)),
            "wk": np.ascontiguousarray(np.asarray(Wk, np.float32)[:, perm].astype(bf16)),
            "wv": np.ascontiguousarray(np.asarray(Wv, np.float32)[:, vcols].astype(bf16)),
            "bq": np.ascontiguousarray(
                np.asarray(bq, np.float32)[perm].reshape(128, 1)),
            "bk": np.ascontiguousarray(
                np.asarray(bk, np.float32)[perm].reshape(128, 1)),
            "bv": np.ascontiguousarray(
                np.asarray(bv, np.float32)[vcols].reshape(128, 1)),
            "wo": Wo_c,
            "bo": bo_c,
        }
        in_maps.append(m)
    return in_maps


def kernel(x, positions, Wq, bq, Wk, bk, Wv, bv, Wo, bo,
           _trace=False, _tmpdir=None):
    nc = _get_nc()
    in_maps = _make_in_maps(x, positions, Wq, bq, Wk, bk, Wv, bv, Wo, bo)
    res = run_bass_kernel_spmd(nc, in_maps, list(range(NCORES)),
                               trace=_trace, tmpdir=_tmpdir)
    full_T = np.empty((E, NT), np.float32)
    for c in range(NCORES):
        full_T[:, 512 * c:512 * (c + 1)] = res.results[c]["out"]
    out = full_T.T.reshape(B, S, E).copy()
    if _trace:
        kernel._last_result = res
    return out


# revision 24
# speedup vs baseline: 1.0731x; 1.0185x over previous
"""Multi-head self-attention with RoPE on 8 Trainium2 NeuronCores.

Sharding: tensor-parallel over the 16 heads (2 heads per core) for the
QKV projections + attention, then an AllToAll that re-shards by token so
each core runs the output projection for its 512-token block.

QKV/Wo projections run as float32r (full-rate fp32 on the PE array,
~1e-4 rel); the attention matmuls (QK^T and PV) run in bf16 so the PE
array stays dense (fast FWL weight loads) and hot. Softmax skips the
max-subtraction (scores/8 stay in [-8, 8] for inputs with unit-variance
activations and 1/sqrt(E)-scaled weights) and gets its denominators for
free from an appended ones-row in the PV matmul. RoPE cos/sin come from
the integer positions with a Cody-Waite range reduction + the ACT
engine's Sin spline.
"""

import sys

for _p in ("/opt/trn_rl_repo", "/opt/pypackages"):
    if _p not in sys.path:
        sys.path.append(_p)

import numpy as np

import concourse.bass as bass
import concourse.mybir as mybir
import concourse.tile as tile
from concourse.bass_utils import run_bass_kernel_spmd
import bass_rust

A = mybir.AluOpType
F32 = mybir.dt.float32
F32R = mybir.dt.float32r
BF16 = mybir.dt.bfloat16
AF = mybir.ActivationFunctionType

B, S, E, H, D = 2, 2048, 1024, 16, 64
NT = B * S            # 4096 tokens, batch-major
NCORES = 8

TWO_PI = 2 * np.pi
INV2PI = float(np.float32(1.0 / TWO_PI))
MAGIC = 12582912.0    # 1.5 * 2^23: add+sub rounds fp32 to nearest int
C1 = 6.28125          # 2*pi split: C1 exact in fp32 with short mantissa
C2 = float(np.float32(TWO_PI - C1))
PI = float(np.pi)
HALF_PI = float(np.pi / 2)

# pair order: head-major so each head's two batches finish together
PAIR_BH = [(0, 0), (1, 0), (0, 1), (1, 1)]


def _split_multisync(nc, max_waits=1, max_updates=1):
    """This container's walrus accepts at most one sync-wait and one
    sync-update per instruction; split extras onto adjacent NoOps."""
    ctr = 0
    for f in nc.m.functions:
        for bb in f.blocks:
            new_list = []
            changed = False
            for ins in bb.instructions:
                si = ins.sync_info
                pre, post = [], []
                if si is not None:
                    waits = list(si.on_wait) if si.on_wait else []
                    if len(waits) > max_waits:
                        for w in waits[:-max_waits]:
                            ctr += 1
                            nop = bass_rust.InstNoOp(
                                name=f"I-mws-{ctr}", ins=[], outs=[])
                            nop.engine = ins.engine
                            nop.sync_info = bass_rust.SyncInfo(
                                on_wait=[w], on_update=[])
                            pre.append(nop)
                        si.on_wait = waits[-max_waits:]
                    upds = list(si.on_update) if si.on_update else []
                    if len(upds) > max_updates:
                        si.on_update = upds[:max_updates]
                        for u in upds[max_updates:]:
                            ctr += 1
                            nop = bass_rust.InstNoOp(
                                name=f"I-mus-{ctr}", ins=[], outs=[])
                            nop.engine = ins.engine
                            nop.sync_info = bass_rust.SyncInfo(
                                on_wait=[], on_update=[u])
                            post.append(nop)
                if pre or post:
                    changed = True
                new_list.extend(pre)
                new_list.append(ins)
                new_list.extend(post)
            if changed:
                bb.instructions = new_list


def _build_nc():
    nc = bass.Bass()

    xT = nc.declare_dram_parameter("xT", [E, NT], BF16, isOutput=False)
    wq = nc.declare_dram_parameter("wq", [E, 128], BF16, isOutput=False)
    wk = nc.declare_dram_parameter("wk", [E, 128], BF16, isOutput=False)
    wv = nc.declare_dram_parameter("wv", [E, 128], BF16, isOutput=False)
    bqp = nc.declare_dram_parameter("bq", [128, 1], F32, isOutput=False)
    bkp = nc.declare_dram_parameter("bk", [128, 1], F32, isOutput=False)
    bvp = nc.declare_dram_parameter("bv", [128, 1], F32, isOutput=False)
    wo = nc.declare_dram_parameter("wo", [E, E], F32R, isOutput=False)
    bop = nc.declare_dram_parameter("bo", [8, 128], F32, isOutput=False)
    posf = nc.declare_dram_parameter("posf", [1, NT], F32, isOutput=False)
    thetap = nc.declare_dram_parameter("theta", [128, 1], F32, isOutput=False)
    identp = nc.declare_dram_parameter("ident", [128, 128], F32,
                                       isOutput=False)
    outp = nc.declare_dram_parameter("out", [E, NT // NCORES], F32,
                                     isOutput=True)

    # per-head-half AllToAll payloads (bf16): block g carries this core's
    # head (A|B) ctx dims for global tokens 512g..512g+512
    sendA = nc.dram_tensor("sendA", [NCORES, 64, 512], BF16)
    recvA = nc.dram_tensor("recvA", [NCORES, 64, 512], BF16)
    sendB = nc.dram_tensor("sendB", [NCORES, 64, 512], BF16)
    recvB = nc.dram_tensor("recvB", [NCORES, 64, 512], BF16)

    with tile.TileContext(nc) as tc:
        with tc.tile_pool(name="const", bufs=1) as cst, \
             tc.tile_pool(name="qmkm", bufs=1) as qmkm, \
             tc.tile_pool(name="vnat", bufs=1) as vnp, \
             tc.tile_pool(name="wop", bufs=1) as wop:
            th = cst.tile([128, 1], F32)
            nc.sync.dma_start(th[:], thetap[:])
            bq_t = cst.tile([128, 1], F32)
            nc.sync.dma_start(bq_t[:], bqp[:])
            bk_t = cst.tile([128, 1], F32)
            nc.sync.dma_start(bk_t[:], bkp[:])
            bv_t = cst.tile([128, 1], F32)
            nc.sync.dma_start(bv_t[:], bvp[:])
            ident = cst.tile([128, 128], F32)
            nc.sync.dma_start(ident[:], identp[:])
            onecol = cst.tile([128, 1], F32)
            nc.vector.memset(onecol[:], 1.0)

            Qm = qmkm.tile([128, NT], BF16)
            Km = qmkm.tile([128, NT], BF16)
            # V token-major with a ones column per head:
            # 32 token-blocks x (64 headA | 1 | 64 headB | 1) columns
            Vna = vnp.tile([128, 32 * 130], BF16)
            vna_v = Vna[:].rearrange("p (g h d) -> p g h d", g=32, h=2)
            nc.vector.tensor_copy(
                vna_v[:, :, :, 64:65],
                onecol[:, 0:1].unsqueeze(1).unsqueeze(1)
                .broadcast_to([128, 32, 2, 1]))

            if True:
                with tc.tile_pool(name="qk01", bufs=1) as qkp, \
                     tc.tile_pool(name="trigp", bufs=1) as trg, \
                     tc.tile_pool(name="wts", bufs=1) as wtp, \
                     tc.tile_pool(name="xr", bufs=2) as xrp, \
                     tc.tile_pool(name="vt", bufs=1) as vtp, \
                     tc.tile_pool(name="ps_proj", bufs=2, space="PSUM") as psp, \
                     tc.tile_pool(name="ps_vt", bufs=2, space="PSUM") as pvt:
                    # rows 0:64 = Q {Ax0,Bx0}/{Ax1,Bx1}, rows 64:128 = K
                    QK0 = qkp.tile([128, NT], F32)
                    QK1 = qkp.tile([128, NT], F32)
                    ones_r = trg.tile([1, 128], F32)
                    nc.vector.memset(ones_r[:], 1.0)
                    w_tiles = {}
                    for name, wsrc in (("q", wq), ("k", wk), ("v", wv)):
                        wr = wtp.tile([128, 8, 128], BF16, tag=f"w{name}")
                        nc.sync.dma_start(
                            wr[:], wsrc[:].rearrange("(a p) d -> p a d",
                                                     p=128))
                        w_tiles[name] = wr
                    VT = vtp.tile([128, NT], F32)
                    for t in range(8):
                        xr = xrp.tile([128, 8, 512], BF16, tag="xr")
                        nc.sync.dma_start(
                            xr[:],
                            xT[:, 512 * t:512 * (t + 1)].rearrange(
                                "(a p) n -> p a n", p=128))
                        cols = slice(512 * t, 512 * (t + 1))
                        for name, bias in (("q", bq_t), ("k", bk_t),
                                           ("v", bv_t)):
                            acc = psp.tile([128, 512], F32, tag="proj")
                            for e in range(8):
                                nc.tensor.matmul(
                                    acc[:], w_tiles[name][:, e, :],
                                    xr[:, e, :],
                                    start=(e == 0), stop=(e == 7))
                            if name == "v":
                                nc.scalar.activation(
                                    VT[:, cols], acc[:], AF.Identity,
                                    bias=bias[:])
                            else:
                                ro = 0 if name == "q" else 64
                                nc.scalar.activation(
                                    QK0[ro:ro + 64, cols], acc[0:64, :],
                                    AF.Identity, bias=bias[0:64, :])
                                nc.scalar.activation(
                                    QK1[ro:ro + 64, cols], acc[64:128, :],
                                    AF.Identity, bias=bias[64:128, :])
                        if t not in (3, 7):
                            continue
                        # half of the tokens is ready: V transpose + RoPE
                        # for these columns while the other half projects
                        hb = 0 if t == 3 else 1
                        hc = slice(2048 * hb, 2048 * (hb + 1))
                        # trig for this half: ang = pos*theta, Cody-Waite
                        # range-reduce, Sin via ACT spline
                        cos_t = trg.tile([128, 2048], F32, tag="cosh",
                                         name=f"cos{hb}")
                        sin_t = trg.tile([128, 2048], F32, tag="sinh",
                                         name=f"sin{hb}")
                        ang = trg.tile([128, 2048], F32, tag="tang",
                                       name=f"ang{hb}")
                        k_t = trg.tile([128, 2048], F32, tag="tk",
                                       name=f"tk{hb}")
                        t1 = trg.tile([128, 2048], F32, tag="tt1",
                                      name=f"tt1{hb}")
                        red = trg.tile([128, 2048], F32, tag="tred",
                                       name=f"tred{hb}")
                        pos_sb = trg.tile([1, 2048], F32, tag="pos",
                                          name=f"pos{hb}")
                        nc.sync.dma_start(
                            pos_sb[:], posf[:, 2048 * hb:2048 * (hb + 1)])
                        with tc.tile_pool(name=f"ps_ang{hb}", bufs=1,
                                          space="PSUM") as psa:
                            pb = psa.tile([128, 2048], F32, tag="angp",
                                          name=f"angp{hb}")
                            for j in range(4):
                                nc.tensor.matmul(
                                    pb[:, 512 * j:512 * (j + 1)], ones_r[:],
                                    pos_sb[:, 512 * j:512 * (j + 1)],
                                    start=True, stop=True)
                            nc.vector.tensor_scalar_mul(ang[:], pb[:], th[:])
                        nc.vector.tensor_scalar(
                            k_t[:], ang[:], INV2PI, MAGIC, A.mult, A.add)
                        nc.vector.tensor_scalar_sub(k_t[:], k_t[:], MAGIC)
                        nc.vector.scalar_tensor_tensor(
                            t1[:], k_t[:], -C1, ang[:], A.mult, A.add)
                        nc.vector.scalar_tensor_tensor(
                            red[:], k_t[:], -C2, t1[:], A.mult, A.add)
                        nc.scalar.activation(sin_t[:], red[:], AF.Sin)
                        nc.vector.tensor_scalar_add(t1[:], red[:], HALF_PI)
                        nc.vector.tensor_scalar(k_t[:], t1[:], PI, None,
                                                A.is_gt)
                        nc.vector.scalar_tensor_tensor(
                            ang[:], k_t[:], -TWO_PI, t1[:], A.mult, A.add)
                        nc.scalar.activation(cos_t[:], ang[:], AF.Sin)
                        for g in range(4 * hb, 4 * hb + 4):
                            ptile = pvt.tile([128, 512], F32, tag="vtp")
                            for j in range(4):
                                kb = 4 * g + j
                                nc.tensor.transpose(
                                    ptile[:, 128 * j:128 * (j + 1)],
                                    VT[:, 128 * kb:128 * (kb + 1)],
                                    ident[:])
                            src = ptile[:].rearrange(
                                "p (j h d) -> p j h d", j=4, h=2)
                            nc.vector.tensor_copy(
                                vna_v[:, 4 * g:4 * (g + 1), :, 0:64], src)
                        # RoPE in place: r0 -> QK0, r1 -> QK1
                        sA = trg.tile([128, 2048], F32, tag="tang",
                                      name=f"ra{hb}")
                        sB = trg.tile([128, 2048], F32, tag="tk",
                                      name=f"rb{hb}")
                        sC = trg.tile([128, 2048], F32, tag="tt1",
                                      name=f"rc{hb}")
                        nc.vector.tensor_mul(sA[:], QK0[:, hc], sin_t[:])
                        nc.vector.tensor_mul(sB[:], QK0[:, hc], cos_t[:])
                        nc.vector.tensor_mul(sC[:], QK1[:, hc], sin_t[:])
                        nc.vector.tensor_sub(QK0[:, hc], sB[:], sC[:])
                        nc.vector.tensor_mul(sB[:], QK1[:, hc], cos_t[:])
                        nc.vector.tensor_add(QK1[:, hc], sA[:], sB[:])
                        # merge to head-contiguous layout + bf16 round
                        for dst, ro in ((Qm, 0), (Km, 64)):
                            nc.vector.tensor_copy(dst[0:32, hc],
                                                  QK0[ro:ro + 32, hc])
                            nc.vector.tensor_copy(dst[32:64, hc],
                                                  QK1[ro:ro + 32, hc])
                            nc.vector.tensor_copy(dst[64:96, hc],
                                                  QK0[ro + 32:ro + 64, hc])
                            nc.vector.tensor_copy(dst[96:128, hc],
                                                  QK1[ro + 32:ro + 64, hc])

            # ---- attention ----
            # output projection weights stream in during attention
            wo_r = wop.tile([128, 8, 1024], F32R)
            nc.sync.dma_start(wo_r[:],
                              wo[:].rearrange("(a p) d -> p a d", p=128))
            bo_t = wop.tile([128, 8], F32)
            nc.sync.dma_start(bo_t[:], bop[:].rearrange("e p -> p e"))
            with tc.tile_pool(name="ctxu", bufs=1) as cxp, \
                 tc.tile_pool(name="nrm", bufs=1) as nrm:
                ctxu = [cxp.tile([65, 2048], F32, name=f"ctxu{p}",
                                 tag=f"cx{p}") for p in range(4)]
                ctxb = [nrm.tile([64, NT], BF16, name=f"ctxb{h}",
                                 tag=f"cb{h}") for h in range(2)]
                ones_row = nrm.tile([1, 64], F32)
                nc.vector.memset(ones_row[:], 1.0)
                ones_row_r = nrm.tile([1, 64], F32R)
                nc.vector.tensor_copy(ones_row_r[:], ones_row[:])
                sums_t = [nrm.tile([1, NT], F32, name=f"sums{h}",
                                   tag="sums") for h in range(2)]
                recip_t = [nrm.tile([1, NT], F32R, name=f"recip{h}",
                                    tag="recip") for h in range(2)]
                rep_t = [nrm.tile([64, NT], F32, name=f"rep{h}",
                                  tag="rep") for h in range(2)]
                with tc.tile_pool(name="pT", bufs=4) as ptp, \
                     tc.tile_pool(name="ps_sc", bufs=2, space="PSUM") as pssc, \
                     tc.tile_pool(name="ps_ctx", bufs=1, space="PSUM") as pscx:
                    for p, (b, h) in enumerate(PAIR_BH):
                        base = 2048 * b
                        hr = 64 * h
                        ctx_acc = [pscx.tile([65, 512], F32,
                                             name=f"ctxacc{p}_{q}",
                                             tag=f"ca{q}")
                                   for q in range(4)]
                        def emit_pv(pend):
                            pkb, phalf, ppT = pend
                            pvb = 16 * b + pkb
                            for qq in range(2):
                                q = 2 * phalf + qq
                                nc.tensor.matmul(
                                    ctx_acc[q][:, :],
                                    Vna[:, 130 * pvb + 65 * h:
                                        130 * pvb + 65 * (h + 1)],
                                    ppT[:, 512 * qq:512 * (qq + 1)],
                                    start=(pkb == 0), stop=(pkb == 15))
                        prev = None
                        for kb in range(16):
                            kcol = base + 128 * kb
                            scs = []
                            for half in range(2):
                                sc = pssc.tile([128, 1024], F32, tag="sc",
                                               name=f"sc{p}_{kb}_{half}")
                                for qq in range(2):
                                    q = 2 * half + qq
                                    nc.tensor.matmul(
                                        sc[:, 512 * qq:512 * (qq + 1)],
                                        Km[hr:hr + 64, kcol:kcol + 128],
                                        Qm[hr:hr + 64,
                                           base + 512 * q:
                                           base + 512 * (q + 1)],
                                        start=True, stop=True)
                                scs.append(sc)
                            pTs = []
                            for half in range(2):
                                pT = ptp.tile([128, 1024], BF16, tag="pT",
                                              name=f"pT{p}_{kb}_{half}")
                                nc.scalar.activation(pTs.append(pT) or pT[:],
                                                     scs[half][:], AF.Exp,
                                                     scale=0.125)
                            if prev is not None:
                                pkb, ppTs = prev
                                for half in range(2):
                                    emit_pv((pkb, half, ppTs[half]))
                            prev = (kb, pTs)
                        pkb, ppTs = prev
                        for half in range(2):
                            emit_pv((pkb, half, ppTs[half]))
                        for q in range(4):
                            nc.vector.tensor_copy(
                                ctxu[p][:, 512 * q:512 * (q + 1)],
                                ctx_acc[q][:])
                        nc.vector.tensor_copy(
                            sums_t[h][:, base:base + 2048],
                            ctxu[p][64:65, :])
                        if p % 2 != 1:
                            continue
                        # both batches of head h done: normalize + send
                        lns = nrm.tile([1, NT], F32, name=f"lns{h}",
                                       tag="lns")
                        nc.scalar.activation(lns[:], sums_t[h][:], AF.Ln)
                        nc.scalar.activation(recip_t[h][:], lns[:], AF.Exp,
                                             scale=-1.0)
                        for g in range(8):
                            # borrow a ctx_acc PSUM slot between pairs
                            rp_ = pscx.tile([64, 512], F32, tag=f"ca{g % 4}",
                                            name=f"rp{h}_{g}")
                            nc.tensor.matmul(
                                rp_[:], ones_row_r[:],
                                recip_t[h][:, 512 * g:512 * (g + 1)],
                                start=True, stop=True)
                            nc.vector.tensor_copy(
                                rep_t[h][:, 512 * g:512 * (g + 1)],
                                rp_[:])
                        # pairs for head h are p-1 (b=0) and p (b=1)
                        for pi, bb in ((p - 1, 0), (p, 1)):
                            nc.vector.tensor_mul(
                                ctxb[h][:, 2048 * bb:2048 * (bb + 1)],
                                ctxu[pi][0:64, :],
                                rep_t[h][:, 2048 * bb:2048 * (bb + 1)])
                        send = sendA if h == 0 else sendB
                        for g in range(8):
                            nc.sync.dma_start(
                                send[g], ctxb[h][:, 512 * g:512 * (g + 1)])
                        nc.gpsimd.collective_compute(
                            "AllToAll", A.bypass,
                            replica_groups=[list(range(NCORES))],
                            ins=[send[:].opt()],
                            outs=[(recvA if h == 0 else recvB)[:].opt()])

            # ---- output projection for this core's 512-token block ----
            # head-A contribution right after the first AllToAll so the PE
            # works while the second AllToAll is in flight
            with tc.tile_pool(name="wrhs", bufs=1) as wrp, \
                 tc.tile_pool(name="ps_o", bufs=1, space="PSUM") as pso:
                rhs_b = wrp.tile([128, 8, 512], BF16)
                rhs_r = wrp.tile([128, 8, 512], F32R)
                outsb = wrp.tile([128, 8, 512], F32)
                po = [pso.tile([128, 512], F32, tag=f"po{eo}",
                               name=f"po{eo}") for eo in range(8)]
                for e in range(8):
                    nc.sync.dma_start(rhs_b[0:64, e, :], recvA[e])
                nc.vector.tensor_copy(rhs_r[0:64, :, :], rhs_b[0:64, :, :])
                for eo in range(8):
                    for e in range(8):
                        nc.tensor.matmul(
                            po[eo][:], wo_r[0:64, e, 128 * eo:128 * (eo + 1)],
                            rhs_r[0:64, e, :],
                            start=(e == 0), stop=False)
                for e in range(8):
                    nc.sync.dma_start(rhs_b[64:128, e, :], recvB[e])
                nc.vector.tensor_copy(rhs_r[64:128, :, :],
                                      rhs_b[64:128, :, :])
                for eo in range(8):
                    for e in range(8):
                        nc.tensor.matmul(
                            po[eo][:],
                            wo_r[64:128, e, 128 * eo:128 * (eo + 1)],
                            rhs_r[64:128, e, :],
                            start=False, stop=(e == 7))
                    nc.scalar.activation(outsb[:, eo, :], po[eo][:],
                                         AF.Identity,
                                         bias=bo_t[:, eo:eo + 1])
                    nc.sync.dma_start(outp[128 * eo:128 * (eo + 1), :],
                                      outsb[:, eo, :])

    nc.finalize()
    _split_multisync(nc)
    return nc


_NC_CACHE = {}


def _get_nc(debug=False):
    if debug not in _NC_CACHE:
        _NC_CACHE[debug] = _build_nc()
    return _NC_CACHE[debug]


def _make_in_maps(x, positions, Wq, bq, Wk, bk, Wv, bv, Wo, bo):
    import ml_dtypes
    bf16 = ml_dtypes.bfloat16
    x = np.ascontiguousarray(np.asarray(x, dtype=np.float32))
    positions = np.asarray(positions)
    xT = np.ascontiguousarray(x.reshape(NT, E).T.astype(bf16))  # [E, NT]
    posf = np.ascontiguousarray(
        positions.astype(np.float32).reshape(1, NT))
    i = np.arange(D // 2)
    theta32 = (10000.0 ** (-2.0 * i / D)).astype(np.float32)
    theta = np.ascontiguousarray(np.tile(theta32, 4).reshape(128, 1))
    ident = np.eye(128, dtype=np.float32)
    Wo_c = np.ascontiguousarray(np.asarray(Wo, dtype=np.float32))
    bo_c = np.ascontiguousarray(
        np.asarray(bo, dtype=np.float32).reshape(8, 128))

    in_maps = []
    ar32 = np.arange(32)
    for c in range(NCORES):
        hA, hB = 2 * c, 2 * c + 1
        perm = np.concatenate([
            64 * hA + 2 * ar32, 64 * hB + 2 * ar32,
            64 * hA + 2 * ar32 + 1, 64 * hB + 2 * ar32 + 1])
        vcols = np.concatenate([64 * hA + np.arange(64),
                                64 * hB + np.arange(64)])
        m = {
            "xT": xT,
            "posf": posf,
            "theta": theta,
            "ident": ident,
            "wq": np.ascontiguousarray(np.asarray(Wq, np.float32)[:, perm].astype(bf16)),
            "wk": np.ascontiguousarray(np.asarray(Wk, np.float32)[:, perm].astype(bf16)),
            "wv": np.ascontiguousarray(np.asarray(Wv, np.float32)[:, vcols].astype(bf16)),
            "bq": np.ascontiguousarray(
                np.asarray(bq, np.float32)[perm].reshape(128, 1)),
            "bk": np.ascontiguousarray(
                np.asarray(bk, np.float32)[perm].reshape(128, 1)),
            "bv": np.ascontiguousarray(
                np.asarray(bv, np.float32)[vcols].reshape(128, 1)),
            "wo": Wo_c,
            "bo": bo_c,
        }
        in_maps.append(m)
    return in_maps


def kernel(x, positions, Wq, bq, Wk, bk, Wv, bv, Wo, bo,
           _trace=False, _tmpdir=None):
    nc = _get_nc()
    in_maps = _make_in_maps(x, positions, Wq, bq, Wk, bk, Wv, bv, Wo, bo)
    res = run_bass_kernel_spmd(nc, in_maps, list(range(NCORES)),
                               trace=_trace, tmpdir=_tmpdir)
    full_T = np.empty((E, NT), np.float32)
    for c in range(NCORES):
        full_T[:, 512 * c:512 * (c + 1)] = res.results[c]["out"]
    out = full_T.T.reshape(B, S, E).copy()
    if _trace:
        kernel._last_result = res
    return out
